# revision 19
# baseline (speedup 1.0000x reference)
"""GNN message-passing kernel for Trainium2 (8 NeuronCores, SPMD) — v6.

out = where(in_deg > 0, segment_sum(hidden[src], dst), hidden)
N=100000 nodes, E=1600000 edges, D=32 (hardcoded).

Design: edges sharded by dst range (core k owns rows [k*12500,(k+1)*12500)).
Cells = (dst block of 256 rows, src chunk of 25000 rows), ordered block-major
so a block's 4 chunks accumulate into one PSUM tile [64, 256] via a single
matmul chain (PSUM pre-zeroed on the Pool engine, all matmuls accumulate).
Messages are fetched with dma_gather (256B bf16 hi/lo rows; gather indices
and dst offsets are PRELOADED into persistent SBUF — no per-cell input DMAs,
which would otherwise halve the SWDGE gather rate).  Tokens are dst-sorted
inside each cell, so a 128-token slot spans only ~42 consecutive dst rows;
the one-hot is built only over each slot's actual span (compile-time-known,
max over cores) and the matmul writes the matching PSUM sub-range:
    psum[64, lo:lo+w] += msg[128tok, 64].T @ onehot[128tok, w]
Per-cell capacities come from the actual inputs at first call (the program
is input-specialized and cached).  The isolated-node fixup adds host-
premasked hidden rows (zero where in-degree > 0).  Phase B (hi+lo combine,
transpose, fixup add) is fused per block right after its PSUM chain closes.
"""

import os
import sys

import numpy as np

for _p in ("/opt/trn_rl_repo", os.path.expanduser("~/.axon_site/_ro/trn_rl_repo")):
    if os.path.isdir(_p) and _p not in sys.path:
        sys.path.insert(0, _p)

import ml_dtypes
import concourse.bacc as bacc
import concourse.mybir as mybir
from concourse import tile
from concourse.bass_utils import run_bass_kernel_spmd

N_NODES = 100000
N_EDGES = 1600000
D = 32
TROW = 128                    # bf16 table row: 32 hi | 32 lo | 0... (256B)
MCOL = 64                     # used message columns (32 hi + 32 lo)

N_CORES = 8
DST_PER_CORE = N_NODES // N_CORES          # 12500
N_CHUNKS = 4
CHUNK = N_NODES // N_CHUNKS                # 25000 (int16 gather index limit)

BLK = 256                     # dst rows per psum block
NBLK = 49                     # ceil(12500/256)
NCELL = NBLK * N_CHUNKS       # 196 cells, ordered (block, chunk)
OUT_ROWS = 12544              # 98*128 rows of output (12500 real + 44 junk)
NT = OUT_ROWS // 128          # 98 half-blocks

MSG_BUFS = 16

_cached = {}


def _build_program(spec, loop_r=None, timing_mode=False, ablate=()):
    """spec: dict with per-cell 'caps' (mult of 16) and per-slot
    'slot_lo'/'slot_w' lists (indexed [cell][slot]).
    ablate: subset of {"gather", "onehot", "matmul", "phaseb"} to skip."""
    caps = spec["caps"]
    slot_lo = spec["slot_lo"]
    slot_w = spec["slot_w"]
    smax = max((c + 127) // 128 for c in caps)
    capmax = max(caps)
    wmax = max(max(ws) for ws in slot_w)

    nc = bacc.Bacc(None, target_bir_lowering=False, debug=False,
                   num_swdge_queues=4,
                   dynamic_dma_scratch_size=spec.get("scratch", 65536))
    f32 = mybir.dt.float32
    bf16 = mybir.dt.bfloat16
    i16 = mybir.dt.int16

    tab_d = nc.dram_tensor("tab", [N_NODES, TROW], bf16,
                           kind="Internal" if timing_mode else "ExternalInput")
    hidm_d = nc.dram_tensor("hidm", [OUT_ROWS, D], f32, kind="ExternalInput")
    sidx_d = nc.dram_tensor("sidx", [NCELL, 128, capmax // 16], i16,
                            kind="ExternalInput")
    doff_d = nc.dram_tensor("doff", [NCELL, 128, smax], bf16,
                            kind="ExternalInput")
    iota2_d = nc.dram_tensor("iota2", [128, BLK], bf16, kind="ExternalInput")
    idn_d = nc.dram_tensor("idn", [MCOL, MCOL], f32, kind="ExternalInput")
    y_d = nc.dram_tensor("y", [OUT_ROWS, D], f32, kind="ExternalOutput")

    with tile.TileContext(nc) as tc:
        with (
            tc.tile_pool(name="cst", bufs=1) as cst_pool,
            tc.tile_pool(name="msg", bufs=MSG_BUFS) as msg_pool,
            tc.tile_pool(name="oh", bufs=8) as oh_pool,
            tc.tile_pool(name="ps", bufs=4, space="PSUM") as ps_pool,
            tc.tile_pool(name="ps2", bufs=2, space="PSUM") as ps2_pool,
            tc.tile_pool(name="fix", bufs=1) as fix_pool,
            tc.tile_pool(name="sb", bufs=3) as sb_pool,
        ):
            iota2_t = cst_pool.tile([128, BLK], bf16)
            idn_t = cst_pool.tile([MCOL, MCOL], f32)
            nc.sync.dma_start(iota2_t[:], iota2_d[:])
            nc.sync.dma_start(idn_t[:], idn_d[:])
            # preload ALL gather indices and dst offsets into persistent SBUF
            sidx_all = cst_pool.tile([128, NCELL, capmax // 16], i16)
            nc.sync.dma_start(
                sidx_all[:], sidx_d.ap().rearrange("g p w -> p g w"))
            doff_all = cst_pool.tile([128, NCELL, smax], bf16)
            nc.scalar.dma_start(
                doff_all[:], doff_d.ap().rearrange("g p s -> p g s"))
            hid_t = fix_pool.tile([128, NT, D], f32)
            nc.sync.dma_start(
                hid_t[:], hidm_d.ap().rearrange("(t p) e -> p t e", p=128))
            y_t = fix_pool.tile([128, NT, D], f32)
            # one-time init of the msg buf ring: partially-gathered tail
            # slots must never expose NaN bit patterns to the matmul.
            for _b in range(MSG_BUFS):
                mz = msg_pool.tile([128, smax, TROW], bf16, tag="msg")
                nc.gpsimd.memset(mz[:], 0.0)

            oh_c = None
            if "onehot" in ablate:
                oh_c = cst_pool.tile([128, smax, wmax], bf16)
                nc.gpsimd.memset(oh_c[:], 0.01)

            def batch_phase(_i=None):
                for b in range(NBLK):
                    do_mm = "matmul" not in ablate
                    if do_mm:
                        ps_t = ps_pool.tile([MCOL, BLK], f32, tag="ps")
                        nc.vector.memset(ps_t[:], 0.0)
                    cells = [b * N_CHUNKS + c for c in range(N_CHUNKS)]
                    last = [g for g in cells if caps[g] > 0]
                    for c in range(N_CHUNKS):
                        g = b * N_CHUNKS + c
                        cap = caps[g]
                        ns = (cap + 127) // 128
                        wcell = max(slot_w[g])
                        msg_t = msg_pool.tile([128, smax, TROW], bf16,
                                              tag="msg")
                        if "gather" not in ablate:
                            nc.gpsimd.dma_gather(
                                msg_t[:, 0:ns, :],
                                tab_d[c * CHUNK:(c + 1) * CHUNK, :],
                                sidx_all[:, g, 0:cap // 16], cap, cap, TROW,
                                single_packet=False, queue_num=g % 4)
                        if "onehot" not in ablate:
                            oh_t = oh_pool.tile([128, smax, wmax], bf16,
                                                tag="oh")
                            nc.vector.tensor_tensor(
                                oh_t[:, 0:ns, 0:wcell],
                                doff_all[:, g, 0:ns].unsqueeze(2)
                                    .broadcast_to([128, ns, wcell]),
                                iota2_t[:, 0:wcell].unsqueeze(1)
                                    .broadcast_to([128, ns, wcell]),
                                mybir.AluOpType.is_equal)
                        else:
                            oh_t = oh_c
                        if do_mm:
                            for k in range(ns):
                                lo, w = slot_lo[g][k], slot_w[g][k]
                                nc.tensor.matmul(
                                    ps_t[:, lo:lo + w], msg_t[:, k, 0:MCOL],
                                    oh_t[:, k, 0:w],
                                    start=False,
                                    stop=(g == last[-1] and k == ns - 1))
                    if not do_mm or "phaseb" in ablate:
                        continue
                    # phase B for this block: combine hi+lo, transpose,
                    # add premasked hidden, stage into y_t
                    sb_t = sb_pool.tile([MCOL, BLK], f32, tag="sbb")
                    nc.vector.tensor_copy(sb_t[:], ps_t[:])
                    ps2_t = ps2_pool.tile([128, 2, MCOL], f32, tag="tr")
                    for t in range(2):
                        nc.tensor.transpose(
                            ps2_t[:, t, :], sb_t[:, t * 128:(t + 1) * 128],
                            idn_t[:])
                    sb2_t = sb_pool.tile([128, 2, MCOL], f32, tag="sb2")
                    nc.vector.tensor_copy(sb2_t[:], ps2_t[:])
                    for t in range(2):
                        j = 2 * b + t
                        nc.vector.tensor_add(y_t[:, j, :],
                                             sb2_t[:, t, 0:32],
                                             sb2_t[:, t, 32:64])
                        nc.vector.tensor_add(y_t[:, j, :], y_t[:, j, :],
                                             hid_t[:, j, :])

            if loop_r is None:
                batch_phase()
            else:
                with tc.For_i(0, loop_r, 1) as _i:
                    batch_phase(_i)

            nc.sync.dma_start(
                y_d.ap().rearrange("(t p) e -> p t e", p=128), y_t[:])

    nc.compile()
    return nc


def _prep_inputs(hidden, src, dst):
    """Returns (spec, in_maps)."""
    src = np.ascontiguousarray(np.asarray(src).astype(np.int64))
    dst = np.ascontiguousarray(np.asarray(dst).astype(np.int64))
    hidden = np.asarray(hidden, np.float32)

    hi = hidden.astype(ml_dtypes.bfloat16)
    lo = (hidden - hi.astype(np.float32)).astype(ml_dtypes.bfloat16)
    tab = np.zeros((N_NODES, TROW), ml_dtypes.bfloat16)
    tab[:, 0:32] = hi
    tab[:, 32:64] = lo

    owner = dst // DST_PER_CORE
    ld = dst - owner * DST_PER_CORE
    block = ld // BLK
    boff = (ld - block * BLK).astype(np.int64)       # dst offset in block
    chunk = src // CHUNK
    cell = (owner * NBLK + block) * N_CHUNKS + chunk

    # sort edges by (cell, dst, src): dst-major for narrow slot spans,
    # src-minor for gather address locality within a dst
    order = np.lexsort((src, boff, cell))
    sc = cell[order]
    counts = np.bincount(sc, minlength=N_CORES * NCELL)
    cs = np.concatenate(([0], np.cumsum(counts)[:-1]))
    rank = np.arange(len(sc)) - np.repeat(cs, counts)

    caps = counts.reshape(N_CORES, NCELL).max(axis=0)
    caps = ((caps + 15) // 16 * 16).astype(np.int64)
    np.maximum(caps, 16, out=caps)
    capmax = int(caps.max())
    smax = int(((caps + 127) // 128).max())

    # per-(cell, slot) dst span: min/max offset across cores
    e_owner = owner[order]
    e_cell = sc - e_owner * NCELL
    e_boff = boff[order]
    slot_of = rank // 128
    gs = (e_cell * smax + slot_of).astype(np.int64)
    lo_arr = np.full(NCELL * smax, BLK, np.int64)
    hi_arr = np.full(NCELL * smax, -1, np.int64)
    np.minimum.at(lo_arr, gs, e_boff)
    np.maximum.at(hi_arr, gs, e_boff)

    slot_lo, slot_w = [], []
    for g in range(NCELL):
        ns = (int(caps[g]) + 127) // 128
        los, ws = [], []
        for k in range(ns):
            l, h = lo_arr[g * smax + k], hi_arr[g * smax + k]
            if h < 0:          # slot holds only padding
                l, h = 0, 0
            w = int(h - l + 1)
            w = min((w + 15) // 16 * 16, BLK)
            l = int(min(l, BLK - w))
            los.append(l)
            ws.append(w)
        slot_lo.append(los)
        slot_w.append(ws)

    src16 = np.zeros((N_CORES, NCELL, capmax), np.int16)
    doff = np.full((N_CORES, NCELL, smax * 128), -1.0, ml_dtypes.bfloat16)
    lo_of_tok = np.array(
        [slot_lo[g][k] for g in range(NCELL)
         for k in range((int(caps[g]) + 127) // 128)], np.int64)
    # map each edge to its slot's lo
    gk_index = {}
    pos = 0
    for g in range(NCELL):
        for k in range((int(caps[g]) + 127) // 128):
            gk_index[g * smax + k] = pos
            pos += 1
    gk_pos = np.array([gk_index[int(x)] for x in gs], np.int64)
    rel = e_boff - lo_of_tok[gk_pos]
    assert rel.min() >= 0 and (rel < np.array(
        [slot_w[g][k] for g in range(NCELL)
         for k in range((int(caps[g]) + 127) // 128)],
        np.int64)[gk_pos]).all()

    src16[e_owner, e_cell, rank] = (src[order] - chunk[order] * CHUNK).astype(
        np.int16)
    doff[e_owner, e_cell, rank] = rel.astype(np.float32).astype(
        ml_dtypes.bfloat16)

    # gather idx layout: token t -> [t % 16, t // 16], replicated x8
    w_ = src16.reshape(N_CORES, NCELL, capmax // 16, 16)
    w_ = np.ascontiguousarray(np.moveaxis(w_, -1, -2))
    src16w = np.tile(w_, (1, 1, 8, 1))
    # doff layout: token t -> [t % 128, t // 128]
    doffw = np.ascontiguousarray(
        np.moveaxis(doff.reshape(N_CORES, NCELL, smax, 128), -1, -2))

    iota2 = np.tile(np.arange(BLK, dtype=np.float32).astype(
        ml_dtypes.bfloat16)[None, :], (128, 1))
    idn = np.eye(MCOL, dtype=np.float32)

    deg = np.bincount(dst, minlength=N_NODES)
    hidm_full = np.where((deg == 0)[:, None], hidden, 0.0).astype(np.float32)

    in_maps = []
    for k in range(N_CORES):
        hidm = np.zeros((OUT_ROWS, D), np.float32)
        hidm[:DST_PER_CORE] = hidm_full[k * DST_PER_CORE:(k + 1) * DST_PER_CORE]
        in_maps.append({
            "tab": tab,
            "hidm": hidm,
            "sidx": np.ascontiguousarray(src16w[k]),
            "doff": np.ascontiguousarray(doffw[k]),
            "iota2": iota2,
            "idn": idn,
        })
    spec = {
        "caps": [int(c) for c in caps],
        "slot_lo": slot_lo,
        "slot_w": slot_w,
    }
    return spec, in_maps


def kernel(hidden, src, dst, **run_kwargs):
    spec, in_maps = _prep_inputs(hidden, src, dst)
    key = (tuple(spec["caps"]),
           tuple(tuple(x) for x in spec["slot_lo"]),
           tuple(tuple(x) for x in spec["slot_w"]))
    if _cached.get("key") != key:
        _cached["nc"] = _build_program(spec)
        _cached["key"] = key
    nc = _cached["nc"]
    res = run_bass_kernel_spmd(nc, in_maps, core_ids=list(range(N_CORES)),
                               **run_kwargs)
    out = np.concatenate(
        [res.results[k]["y"][:DST_PER_CORE] for k in range(N_CORES)], axis=0)
    if run_kwargs:
        _cached["last_results"] = res
    return out


# revision 28
# speedup vs baseline: 1.2809x; 1.2809x over previous
"""GNN message-passing kernel for Trainium2 (8 NeuronCores, SPMD) — v6.

out = where(in_deg > 0, segment_sum(hidden[src], dst), hidden)
N=100000 nodes, E=1600000 edges, D=32 (hardcoded).

Design: edges sharded by dst range (core k owns rows [k*12500,(k+1)*12500)).
Cells = (dst block of 256 rows, src chunk of 25000 rows), ordered block-major
so a block's 4 chunks accumulate into one PSUM tile [64, 256] via a single
matmul chain (PSUM pre-zeroed on the Pool engine, all matmuls accumulate).
Messages are fetched with dma_gather (256B bf16 hi/lo rows; gather indices
and dst offsets are PRELOADED into persistent SBUF — no per-cell input DMAs,
which would otherwise halve the SWDGE gather rate).  Tokens are dst-sorted
inside each cell, so a 128-token slot spans only ~42 consecutive dst rows;
the one-hot is built only over each slot's actual span (compile-time-known,
max over cores) and the matmul writes the matching PSUM sub-range:
    psum[64, lo:lo+w] += msg[128tok, 64].T @ onehot[128tok, w]
Per-cell capacities come from the actual inputs at first call (the program
is input-specialized and cached).  The isolated-node fixup adds host-
premasked hidden rows (zero where in-degree > 0).  Phase B (hi+lo combine,
transpose, fixup add) is fused per block right after its PSUM chain closes.
"""

import os
import sys

import numpy as np

for _p in ("/opt/trn_rl_repo", os.path.expanduser("~/.axon_site/_ro/trn_rl_repo")):
    if os.path.isdir(_p) and _p not in sys.path:
        sys.path.insert(0, _p)

import ml_dtypes
import concourse.bacc as bacc
import concourse.mybir as mybir
from concourse import tile
from concourse.bass_utils import run_bass_kernel_spmd

N_NODES = 100000
N_EDGES = 1600000
D = 32
TROW = 128                    # bf16 table row: 32 hi | 32 lo | 0... (256B)
MCOL = 64                     # used message columns (32 hi + 32 lo)

N_CORES = 8
DST_PER_CORE = N_NODES // N_CORES          # 12500
N_CHUNKS = 4
CHUNK = N_NODES // N_CHUNKS                # 25000 (int16 gather index limit)

BLK = 256                     # dst rows per psum block
NBLK = 49                     # ceil(12500/256)
NCELL = NBLK * N_CHUNKS       # 196 cells, ordered (block, chunk)
OUT_ROWS = 12544              # 98*128 rows of output (12500 real + 44 junk)
NT = OUT_ROWS // 128          # 98 half-blocks

MSG_BUFS = 16

_cached = {}


def _build_program(spec, loop_r=None, timing_mode=False, ablate=()):
    """spec: dict with per-cell 'caps' (mult of 16) and per-slot
    'slot_lo'/'slot_w' lists (indexed [cell][slot]).
    ablate: subset of {"gather", "onehot", "matmul", "phaseb"} to skip."""
    caps = spec["caps"]
    slot_lo = spec["slot_lo"]
    slot_w = spec["slot_w"]
    smax = max((c + 127) // 128 for c in caps)
    capmax = max(caps)
    # sidx row stride in int16s: multiple of 32 so every cell's index array
    # is 64B-aligned inside the persistent SBUF tile (SWDGE gen reads it)
    widx = (capmax // 16 + 31) // 32 * 32
    wmax = max(max(ws) for ws in slot_w)

    nc = bacc.Bacc(None, target_bir_lowering=False, debug=False,
                   num_swdge_queues=4,
                   dynamic_dma_scratch_size=spec.get("scratch", 65536))
    f32 = mybir.dt.float32
    bf16 = mybir.dt.bfloat16
    i16 = mybir.dt.int16

    tab_d = nc.dram_tensor("tab", [N_NODES, TROW], bf16,
                           kind="Internal" if timing_mode else "ExternalInput")
    hidm_d = nc.dram_tensor("hidm", [OUT_ROWS, D], f32, kind="ExternalInput")
    sidx_d = nc.dram_tensor("sidx", [NCELL, 128, widx], i16,
                            kind="ExternalInput")
    doff_d = nc.dram_tensor("doff", [NCELL, 128, smax], bf16,
                            kind="ExternalInput")
    iota2_d = nc.dram_tensor("iota2", [128, BLK], bf16, kind="ExternalInput")
    idn_d = nc.dram_tensor("idn", [MCOL, MCOL], f32, kind="ExternalInput")
    y_d = nc.dram_tensor("y", [OUT_ROWS, D], f32, kind="ExternalOutput")

    with tile.TileContext(nc) as tc:
        with (
            tc.tile_pool(name="cst", bufs=1) as cst_pool,
            tc.tile_pool(name="msg", bufs=MSG_BUFS) as msg_pool,
            tc.tile_pool(name="oh", bufs=8) as oh_pool,
            tc.tile_pool(name="ps", bufs=4, space="PSUM") as ps_pool,
            tc.tile_pool(name="ps2", bufs=2, space="PSUM") as ps2_pool,
            tc.tile_pool(name="fix", bufs=1) as fix_pool,
            tc.tile_pool(name="sb", bufs=3) as sb_pool,
        ):
            iota2_t = cst_pool.tile([128, BLK], bf16)
            idn_t = cst_pool.tile([MCOL, MCOL], f32)
            nc.sync.dma_start(iota2_t[:], iota2_d[:])
            nc.sync.dma_start(idn_t[:], idn_d[:])
            # preload ALL gather indices and dst offsets into persistent SBUF
            sidx_all = cst_pool.tile([128, NCELL, widx], i16)
            nc.sync.dma_start(
                sidx_all[:], sidx_d.ap().rearrange("g p w -> p g w"))
            doff_all = cst_pool.tile([128, NCELL, smax], bf16)
            nc.scalar.dma_start(
                doff_all[:], doff_d.ap().rearrange("g p s -> p g s"))
            hid_t = fix_pool.tile([128, NT, D], f32)
            nc.sync.dma_start(
                hid_t[:], hidm_d.ap().rearrange("(t p) e -> p t e", p=128))
            y_t = fix_pool.tile([128, NT, D], f32)
            nc.vector.memset(y_t[:], 0.0)
            # one-time init of the msg buf ring: partially-gathered tail
            # slots must never expose NaN bit patterns to the matmul.
            for _b in range(MSG_BUFS):
                mz = msg_pool.tile([128, smax, TROW], bf16, tag="msg")
                nc.gpsimd.memset(mz[:], 0.0)

            oh_c = None
            if "onehot" in ablate:
                oh_c = cst_pool.tile([128, smax, wmax], bf16)
                nc.gpsimd.memset(oh_c[:], 0.01)

            def batch_phase(_i=None):
                for b in range(NBLK):
                    do_mm = "matmul" not in ablate
                    if do_mm:
                        ps_t = ps_pool.tile([MCOL, BLK], f32, tag="ps")
                        nc.scalar.memzero(ps_t[:])
                    cells = [b * N_CHUNKS + c for c in range(N_CHUNKS)]
                    last = [g for g in cells if caps[g] > 0]
                    for c in range(N_CHUNKS):
                        g = b * N_CHUNKS + c
                        cap = caps[g]
                        ns = (cap + 127) // 128
                        wcell = max(slot_w[g])
                        msg_t = msg_pool.tile([128, smax, TROW], bf16,
                                              tag="msg")
                        if spec.get("sidxdma"):
                            sidx_t = oh_pool.tile([128, widx], i16,
                                                  tag="sidx", bufs=16)
                            nc.sync.dma_start(sidx_t[:], sidx_d[g])
                            idx_ap = sidx_t[:, 0:cap // 16]
                        else:
                            idx_ap = sidx_all[:, g, 0:cap // 16]
                        if "gather" not in ablate:
                            nc.gpsimd.dma_gather(
                                msg_t[:, 0:ns, :],
                                tab_d[c * CHUNK:(c + 1) * CHUNK, :],
                                idx_ap, cap, cap, TROW,
                                single_packet=False, queue_num=g % 4)
                        if "onehot" not in ablate:
                            oh_t = oh_pool.tile([128, smax, wmax], bf16,
                                                tag="oh")
                            nc.vector.tensor_tensor(
                                oh_t[:, 0:ns, 0:wcell],
                                doff_all[:, g, 0:ns].unsqueeze(2)
                                    .broadcast_to([128, ns, wcell]),
                                iota2_t[:, 0:wcell].unsqueeze(1)
                                    .broadcast_to([128, ns, wcell]),
                                mybir.AluOpType.is_equal)
                        else:
                            oh_t = oh_c
                        if do_mm:
                            for k in range(ns):
                                lo, w = slot_lo[g][k], slot_w[g][k]
                                nc.tensor.matmul(
                                    ps_t[:, lo:lo + w], msg_t[:, k, 0:MCOL],
                                    oh_t[:, k, 0:w],
                                    start=False,
                                    stop=(g == last[-1] and k == ns - 1))
                    if not do_mm or "phaseb" in ablate:
                        continue
                    # phase B for this block: combine hi+lo, transpose,
                    # add premasked hidden, stage into y_t
                    sb_t = sb_pool.tile([MCOL, BLK], f32, tag="sbb")
                    nc.scalar.copy(sb_t[:], ps_t[:])
                    ps2_t = ps2_pool.tile([128, 2, MCOL], f32, tag="tr")
                    for t in range(2):
                        nc.tensor.transpose(
                            ps2_t[:, t, :], sb_t[:, t * 128:(t + 1) * 128],
                            idn_t[:])
                    sb2_t = sb_pool.tile([128, 2, MCOL], f32, tag="sb2")
                    nc.scalar.copy(sb2_t[:], ps2_t[:])
                    for t in range(2):
                        j = 2 * b + t
                        nc.vector.tensor_add(y_t[:, j, :],
                                             sb2_t[:, t, 0:32],
                                             sb2_t[:, t, 32:64])
                        nc.vector.tensor_add(y_t[:, j, :], y_t[:, j, :],
                                             hid_t[:, j, :])

            if loop_r is None:
                batch_phase()
            else:
                with tc.For_i(0, loop_r, 1) as _i:
                    batch_phase(_i)

            nc.sync.dma_start(
                y_d.ap().rearrange("(t p) e -> p t e", p=128), y_t[:])

    nc.compile()
    return nc


def _prep_inputs(hidden, src, dst):
    """Returns (spec, in_maps)."""
    src = np.ascontiguousarray(np.asarray(src).astype(np.int64))
    dst = np.ascontiguousarray(np.asarray(dst).astype(np.int64))
    hidden = np.asarray(hidden, np.float32)

    hi = hidden.astype(ml_dtypes.bfloat16)
    lo = (hidden - hi.astype(np.float32)).astype(ml_dtypes.bfloat16)
    tab = np.zeros((N_NODES, TROW), ml_dtypes.bfloat16)
    tab[:, 0:32] = hi
    tab[:, 32:64] = lo

    owner = dst // DST_PER_CORE
    ld = dst - owner * DST_PER_CORE
    block = ld // BLK
    boff = (ld - block * BLK).astype(np.int64)       # dst offset in block
    chunk = src // CHUNK
    cell = (owner * NBLK + block) * N_CHUNKS + chunk

    # sort edges by (cell, dst, src): dst-major for narrow slot spans,
    # src-minor for gather address locality within a dst
    order = np.lexsort((src, boff, cell))
    sc = cell[order]
    counts = np.bincount(sc, minlength=N_CORES * NCELL)
    cs = np.concatenate(([0], np.cumsum(counts)[:-1]))
    rank = np.arange(len(sc)) - np.repeat(cs, counts)

    caps = counts.reshape(N_CORES, NCELL).max(axis=0)
    caps = ((caps + 15) // 16 * 16).astype(np.int64)
    np.maximum(caps, 16, out=caps)
    capmax = int(caps.max())
    smax = int(((caps + 127) // 128).max())

    # per-(cell, slot) dst span: min/max offset across cores
    e_owner = owner[order]
    e_cell = sc - e_owner * NCELL
    e_boff = boff[order]
    slot_of = rank // 128
    gs = (e_cell * smax + slot_of).astype(np.int64)
    lo_arr = np.full(NCELL * smax, BLK, np.int64)
    hi_arr = np.full(NCELL * smax, -1, np.int64)
    np.minimum.at(lo_arr, gs, e_boff)
    np.maximum.at(hi_arr, gs, e_boff)

    slot_lo, slot_w = [], []
    for g in range(NCELL):
        ns = (int(caps[g]) + 127) // 128
        los, ws = [], []
        for k in range(ns):
            l, h = lo_arr[g * smax + k], hi_arr[g * smax + k]
            if h < 0:          # slot holds only padding
                l, h = 0, 0
            w = int(h - l + 1)
            w = min((w + 15) // 16 * 16, BLK)
            l = int(min(l, BLK - w))
            los.append(l)
            ws.append(w)
        slot_lo.append(los)
        slot_w.append(ws)

    src16 = np.zeros((N_CORES, NCELL, capmax), np.int16)
    doff = np.full((N_CORES, NCELL, smax * 128), -1.0, ml_dtypes.bfloat16)
    lo_of_tok = np.array(
        [slot_lo[g][k] for g in range(NCELL)
         for k in range((int(caps[g]) + 127) // 128)], np.int64)
    # map each edge to its slot's lo
    gk_index = {}
    pos = 0
    for g in range(NCELL):
        for k in range((int(caps[g]) + 127) // 128):
            gk_index[g * smax + k] = pos
            pos += 1
    gk_pos = np.array([gk_index[int(x)] for x in gs], np.int64)
    rel = e_boff - lo_of_tok[gk_pos]
    assert rel.min() >= 0 and (rel < np.array(
        [slot_w[g][k] for g in range(NCELL)
         for k in range((int(caps[g]) + 127) // 128)],
        np.int64)[gk_pos]).all()

    src16[e_owner, e_cell, rank] = (src[order] - chunk[order] * CHUNK).astype(
        np.int16)
    doff[e_owner, e_cell, rank] = rel.astype(np.float32).astype(
        ml_dtypes.bfloat16)

    # gather idx layout: token t -> [t % 16, t // 16], replicated x8;
    # rows padded to a multiple of 32 int16s (64B alignment in SBUF)
    widx = (capmax // 16 + 31) // 32 * 32
    w_ = src16.reshape(N_CORES, NCELL, capmax // 16, 16)
    w_ = np.moveaxis(w_, -1, -2)                    # [C, G, 16, capmax//16]
    w_p = np.zeros((N_CORES, NCELL, 16, widx), np.int16)
    w_p[:, :, :, 0:capmax // 16] = w_
    src16w = np.tile(w_p, (1, 1, 8, 1))
    # doff layout: token t -> [t % 128, t // 128]
    doffw = np.ascontiguousarray(
        np.moveaxis(doff.reshape(N_CORES, NCELL, smax, 128), -1, -2))

    iota2 = np.tile(np.arange(BLK, dtype=np.float32).astype(
        ml_dtypes.bfloat16)[None, :], (128, 1))
    idn = np.eye(MCOL, dtype=np.float32)

    deg = np.bincount(dst, minlength=N_NODES)
    hidm_full = np.where((deg == 0)[:, None], hidden, 0.0).astype(np.float32)

    in_maps = []
    for k in range(N_CORES):
        hidm = np.zeros((OUT_ROWS, D), np.float32)
        hidm[:DST_PER_CORE] = hidm_full[k * DST_PER_CORE:(k + 1) * DST_PER_CORE]
        in_maps.append({
            "tab": tab,
            "hidm": hidm,
            "sidx": np.ascontiguousarray(src16w[k]),
            "doff": np.ascontiguousarray(doffw[k]),
            "iota2": iota2,
            "idn": idn,
        })
    spec = {
        "caps": [int(c) for c in caps],
        "slot_lo": slot_lo,
        "slot_w": slot_w,
    }
    return spec, in_maps


def kernel(hidden, src, dst, **run_kwargs):
    spec, in_maps = _prep_inputs(hidden, src, dst)
    key = (tuple(spec["caps"]),
           tuple(tuple(x) for x in spec["slot_lo"]),
           tuple(tuple(x) for x in spec["slot_w"]))
    if _cached.get("key") != key:
        _cached["nc"] = _build_program(spec)
        _cached["key"] = key
    nc = _cached["nc"]
    res = run_bass_kernel_spmd(nc, in_maps, core_ids=list(range(N_CORES)),
                               **run_kwargs)
    out = np.concatenate(
        [res.results[k]["y"][:DST_PER_CORE] for k in range(N_CORES)], axis=0)
    if run_kwargs:
        _cached["last_results"] = res
    return out


# revision 31
# speedup vs baseline: 1.2860x; 1.0040x over previous
"""GNN message-passing kernel for Trainium2 (8 NeuronCores, SPMD) — v6.

out = where(in_deg > 0, segment_sum(hidden[src], dst), hidden)
N=100000 nodes, E=1600000 edges, D=32 (hardcoded).

Design: edges sharded by dst range (core k owns rows [k*12500,(k+1)*12500)).
Cells = (dst block of 256 rows, src chunk of 25000 rows), ordered block-major
so a block's 4 chunks accumulate into one PSUM tile [64, 256] via a single
matmul chain (PSUM pre-zeroed on the Pool engine, all matmuls accumulate).
Messages are fetched with dma_gather (256B bf16 hi/lo rows; gather indices
and dst offsets are PRELOADED into persistent SBUF — no per-cell input DMAs,
which would otherwise halve the SWDGE gather rate).  Tokens are dst-sorted
inside each cell, so a 128-token slot spans only ~42 consecutive dst rows;
the one-hot is built only over each slot's actual span (compile-time-known,
max over cores) and the matmul writes the matching PSUM sub-range:
    psum[64, lo:lo+w] += msg[128tok, 64].T @ onehot[128tok, w]
Per-cell capacities come from the actual inputs at first call (the program
is input-specialized and cached).  The isolated-node fixup adds host-
premasked hidden rows (zero where in-degree > 0).  Phase B (hi+lo combine,
transpose, fixup add) is fused per block right after its PSUM chain closes.
"""

import os
import sys

import numpy as np

for _p in ("/opt/trn_rl_repo", os.path.expanduser("~/.axon_site/_ro/trn_rl_repo")):
    if os.path.isdir(_p) and _p not in sys.path:
        sys.path.insert(0, _p)

import ml_dtypes
import concourse.bacc as bacc
import concourse.mybir as mybir
from concourse import tile
from concourse.bass_utils import run_bass_kernel_spmd

N_NODES = 100000
N_EDGES = 1600000
D = 32
TROW = 128                    # bf16 table row: 32 hi | 32 lo | 0... (256B)
MCOL = 64                     # used message columns (32 hi + 32 lo)

N_CORES = 8
DST_PER_CORE = N_NODES // N_CORES          # 12500
N_CHUNKS = 4
CHUNK = N_NODES // N_CHUNKS                # 25000 (int16 gather index limit)

BLK = 256                     # dst rows per psum block
NBLK = 49                     # ceil(12500/256)
NCELL = NBLK * N_CHUNKS       # 196 cells, ordered (block, chunk)
OUT_ROWS = 12544              # 98*128 rows of output (12500 real + 44 junk)
NT = OUT_ROWS // 128          # 98 half-blocks

MSG_BUFS = 16

_cached = {}


def _build_program(spec, loop_r=None, timing_mode=False, ablate=()):
    """spec: dict with per-cell 'caps' (mult of 16) and per-slot
    'slot_lo'/'slot_w' lists (indexed [cell][slot]).
    ablate: subset of {"gather", "onehot", "matmul", "phaseb"} to skip."""
    caps = spec["caps"]
    slot_lo = spec["slot_lo"]
    slot_w = spec["slot_w"]
    smax = max((c + 127) // 128 for c in caps)
    capmax = max(caps)
    # sidx row stride in int16s: multiple of 32 so every cell's index array
    # is 64B-aligned inside the persistent SBUF tile (SWDGE gen reads it)
    widx = (capmax // 16 + 31) // 32 * 32
    wmax = max(max(ws) for ws in slot_w)

    nc = bacc.Bacc(None, target_bir_lowering=False, debug=False,
                   num_swdge_queues=4,
                   dynamic_dma_scratch_size=spec.get("scratch", 65536))
    f32 = mybir.dt.float32
    bf16 = mybir.dt.bfloat16
    i16 = mybir.dt.int16

    tab_d = nc.dram_tensor("tab", [N_NODES, TROW], bf16,
                           kind="Internal" if timing_mode else "ExternalInput")
    hidm_d = nc.dram_tensor("hidm", [OUT_ROWS, D], f32, kind="ExternalInput")
    sidx_d = nc.dram_tensor("sidx", [NCELL, 128, widx], i16,
                            kind="ExternalInput")
    doff_d = nc.dram_tensor("doff", [NCELL, 128, smax], bf16,
                            kind="ExternalInput")
    iota2_d = nc.dram_tensor("iota2", [128, BLK], bf16, kind="ExternalInput")
    idn_d = nc.dram_tensor("idn", [MCOL, MCOL], f32, kind="ExternalInput")
    y_d = nc.dram_tensor("y", [OUT_ROWS, D], f32, kind="ExternalOutput")

    with tile.TileContext(nc) as tc:
        with (
            tc.tile_pool(name="cst", bufs=1) as cst_pool,
            tc.tile_pool(name="msg", bufs=MSG_BUFS) as msg_pool,
            tc.tile_pool(name="idx", bufs=16) as idx_pool,
            tc.tile_pool(name="oh", bufs=8) as oh_pool,
            tc.tile_pool(name="ps", bufs=4, space="PSUM") as ps_pool,
            tc.tile_pool(name="ps2", bufs=2, space="PSUM") as ps2_pool,
            tc.tile_pool(name="fix", bufs=1) as fix_pool,
            tc.tile_pool(name="sb", bufs=3) as sb_pool,
        ):
            iota2_t = cst_pool.tile([128, BLK], bf16)
            idn_t = cst_pool.tile([MCOL, MCOL], f32)
            nc.sync.dma_start(iota2_t[:], iota2_d[:])
            nc.sync.dma_start(idn_t[:], idn_d[:])
            # preload dst offsets (and optionally gather indices) into
            # persistent SBUF: no per-cell Act-engine DMAs inside the loop
            if spec.get("sidx_preload"):
                sidx_all = cst_pool.tile([128, NCELL, widx], i16)
                nc.sync.dma_start(
                    sidx_all[:], sidx_d.ap().rearrange("g p w -> p g w"))
            doff_all = cst_pool.tile([128, NCELL, smax], bf16)
            nc.scalar.dma_start(
                doff_all[:], doff_d.ap().rearrange("g p s -> p g s"))
            hid_t = fix_pool.tile([128, NT, D], f32)
            nc.sync.dma_start(
                hid_t[:], hidm_d.ap().rearrange("(t p) e -> p t e", p=128))
            y_t = fix_pool.tile([128, NT, D], f32)
            nc.vector.memset(y_t[:], 0.0)
            # one-time init of the msg buf ring: partially-gathered tail
            # slots must never expose NaN bit patterns to the matmul.
            for _b in range(MSG_BUFS):
                mz = msg_pool.tile([128, smax, TROW], bf16, tag="msg")
                nc.gpsimd.memset(mz[:], 0.0)

            oh_c = None
            if "onehot" in ablate:
                oh_c = cst_pool.tile([128, smax, wmax], bf16)
                nc.gpsimd.memset(oh_c[:], 0.01)

            def batch_phase(_i=None):
                for b in range(NBLK):
                    do_mm = "matmul" not in ablate
                    if do_mm:
                        ps_t = ps_pool.tile([MCOL, BLK], f32, tag="ps")
                        nc.scalar.memzero(ps_t[:])
                    cells = [b * N_CHUNKS + c for c in range(N_CHUNKS)]
                    last = [g for g in cells if caps[g] > 0]
                    for c in range(N_CHUNKS):
                        g = b * N_CHUNKS + c
                        cap = caps[g]
                        ns = (cap + 127) // 128
                        wcell = max(slot_w[g])
                        msg_t = msg_pool.tile([128, smax, TROW], bf16,
                                              tag="msg")
                        if spec.get("sidx_preload"):
                            idx_ap = sidx_all[:, g, 0:cap // 16]
                        else:
                            sidx_t = idx_pool.tile([128, widx], i16,
                                                   tag="sidx")
                            nc.sync.dma_start(sidx_t[:], sidx_d[g])
                            idx_ap = sidx_t[:, 0:cap // 16]
                        if "gather" not in ablate:
                            nc.gpsimd.dma_gather(
                                msg_t[:, 0:ns, :],
                                tab_d[c * CHUNK:(c + 1) * CHUNK, :],
                                idx_ap, cap, cap, TROW,
                                single_packet=False, queue_num=g % 4)
                        if "onehot" not in ablate:
                            oh_t = oh_pool.tile([128, smax, wmax], bf16,
                                                tag="oh")
                            nc.vector.tensor_tensor(
                                oh_t[:, 0:ns, 0:wcell],
                                doff_all[:, g, 0:ns].unsqueeze(2)
                                    .broadcast_to([128, ns, wcell]),
                                iota2_t[:, 0:wcell].unsqueeze(1)
                                    .broadcast_to([128, ns, wcell]),
                                mybir.AluOpType.is_equal)
                        else:
                            oh_t = oh_c
                        if do_mm:
                            for k in range(ns):
                                lo, w = slot_lo[g][k], slot_w[g][k]
                                nc.tensor.matmul(
                                    ps_t[:, lo:lo + w], msg_t[:, k, 0:MCOL],
                                    oh_t[:, k, 0:w],
                                    start=False,
                                    stop=(g == last[-1] and k == ns - 1))
                    if not do_mm or "phaseb" in ablate:
                        continue
                    # phase B for this block: combine hi+lo, transpose,
                    # add premasked hidden, stage into y_t
                    sb_t = sb_pool.tile([MCOL, BLK], f32, tag="sbb")
                    nc.scalar.copy(sb_t[:], ps_t[:])
                    ps2_t = ps2_pool.tile([128, 2, MCOL], f32, tag="tr")
                    for t in range(2):
                        nc.tensor.transpose(
                            ps2_t[:, t, :], sb_t[:, t * 128:(t + 1) * 128],
                            idn_t[:])
                    sb2_t = sb_pool.tile([128, 2, MCOL], f32, tag="sb2")
                    nc.scalar.copy(sb2_t[:], ps2_t[:])
                    for t in range(2):
                        j = 2 * b + t
                        nc.vector.tensor_add(y_t[:, j, :],
                                             sb2_t[:, t, 0:32],
                                             sb2_t[:, t, 32:64])
                        nc.vector.tensor_add(y_t[:, j, :], y_t[:, j, :],
                                             hid_t[:, j, :])

            if loop_r is None:
                batch_phase()
            else:
                with tc.For_i(0, loop_r, 1) as _i:
                    batch_phase(_i)

            nc.sync.dma_start(
                y_d.ap().rearrange("(t p) e -> p t e", p=128), y_t[:])

    nc.compile()
    return nc


def _prep_inputs(hidden, src, dst):
    """Returns (spec, in_maps)."""
    src = np.ascontiguousarray(np.asarray(src).astype(np.int64))
    dst = np.ascontiguousarray(np.asarray(dst).astype(np.int64))
    hidden = np.asarray(hidden, np.float32)

    hi = hidden.astype(ml_dtypes.bfloat16)
    lo = (hidden - hi.astype(np.float32)).astype(ml_dtypes.bfloat16)
    tab = np.zeros((N_NODES, TROW), ml_dtypes.bfloat16)
    tab[:, 0:32] = hi
    tab[:, 32:64] = lo

    owner = dst // DST_PER_CORE
    ld = dst - owner * DST_PER_CORE
    block = ld // BLK
    boff = (ld - block * BLK).astype(np.int64)       # dst offset in block
    chunk = src // CHUNK
    cell = (owner * NBLK + block) * N_CHUNKS + chunk

    # sort edges by (cell, dst, src): dst-major for narrow slot spans,
    # src-minor for gather address locality within a dst
    order = np.lexsort((src, boff, cell))
    sc = cell[order]
    counts = np.bincount(sc, minlength=N_CORES * NCELL)
    cs = np.concatenate(([0], np.cumsum(counts)[:-1]))
    rank = np.arange(len(sc)) - np.repeat(cs, counts)

    caps = counts.reshape(N_CORES, NCELL).max(axis=0)
    caps = ((caps + 15) // 16 * 16).astype(np.int64)
    np.maximum(caps, 16, out=caps)
    capmax = int(caps.max())
    smax = int(((caps + 127) // 128).max())

    # per-(cell, slot) dst span: min/max offset across cores
    e_owner = owner[order]
    e_cell = sc - e_owner * NCELL
    e_boff = boff[order]
    slot_of = rank // 128
    gs = (e_cell * smax + slot_of).astype(np.int64)
    lo_arr = np.full(NCELL * smax, BLK, np.int64)
    hi_arr = np.full(NCELL * smax, -1, np.int64)
    np.minimum.at(lo_arr, gs, e_boff)
    np.maximum.at(hi_arr, gs, e_boff)

    slot_lo, slot_w = [], []
    for g in range(NCELL):
        ns = (int(caps[g]) + 127) // 128
        los, ws = [], []
        for k in range(ns):
            l, h = lo_arr[g * smax + k], hi_arr[g * smax + k]
            if h < 0:          # slot holds only padding
                l, h = 0, 0
            w = int(h - l + 1)
            w = min((w + 15) // 16 * 16, BLK)
            l = int(min(l, BLK - w))
            los.append(l)
            ws.append(w)
        slot_lo.append(los)
        slot_w.append(ws)

    src16 = np.zeros((N_CORES, NCELL, capmax), np.int16)
    doff = np.full((N_CORES, NCELL, smax * 128), -1.0, ml_dtypes.bfloat16)
    lo_of_tok = np.array(
        [slot_lo[g][k] for g in range(NCELL)
         for k in range((int(caps[g]) + 127) // 128)], np.int64)
    # map each edge to its slot's lo
    gk_index = {}
    pos = 0
    for g in range(NCELL):
        for k in range((int(caps[g]) + 127) // 128):
            gk_index[g * smax + k] = pos
            pos += 1
    gk_pos = np.array([gk_index[int(x)] for x in gs], np.int64)
    rel = e_boff - lo_of_tok[gk_pos]
    assert rel.min() >= 0 and (rel < np.array(
        [slot_w[g][k] for g in range(NCELL)
         for k in range((int(caps[g]) + 127) // 128)],
        np.int64)[gk_pos]).all()

    src16[e_owner, e_cell, rank] = (src[order] - chunk[order] * CHUNK).astype(
        np.int16)
    doff[e_owner, e_cell, rank] = rel.astype(np.float32).astype(
        ml_dtypes.bfloat16)

    # gather idx layout: token t -> [t % 16, t // 16], replicated x8;
    # rows padded to a multiple of 32 int16s (64B alignment in SBUF)
    widx = (capmax // 16 + 31) // 32 * 32
    w_ = src16.reshape(N_CORES, NCELL, capmax // 16, 16)
    w_ = np.moveaxis(w_, -1, -2)                    # [C, G, 16, capmax//16]
    w_p = np.zeros((N_CORES, NCELL, 16, widx), np.int16)
    w_p[:, :, :, 0:capmax // 16] = w_
    src16w = np.tile(w_p, (1, 1, 8, 1))
    # doff layout: token t -> [t % 128, t // 128]
    doffw = np.ascontiguousarray(
        np.moveaxis(doff.reshape(N_CORES, NCELL, smax, 128), -1, -2))

    iota2 = np.tile(np.arange(BLK, dtype=np.float32).astype(
        ml_dtypes.bfloat16)[None, :], (128, 1))
    idn = np.eye(MCOL, dtype=np.float32)

    deg = np.bincount(dst, minlength=N_NODES)
    hidm_full = np.where((deg == 0)[:, None], hidden, 0.0).astype(np.float32)

    in_maps = []
    for k in range(N_CORES):
        hidm = np.zeros((OUT_ROWS, D), np.float32)
        hidm[:DST_PER_CORE] = hidm_full[k * DST_PER_CORE:(k + 1) * DST_PER_CORE]
        in_maps.append({
            "tab": tab,
            "hidm": hidm,
            "sidx": np.ascontiguousarray(src16w[k]),
            "doff": np.ascontiguousarray(doffw[k]),
            "iota2": iota2,
            "idn": idn,
        })
    spec = {
        "caps": [int(c) for c in caps],
        "slot_lo": slot_lo,
        "slot_w": slot_w,
    }
    return spec, in_maps


def kernel(hidden, src, dst, **run_kwargs):
    spec, in_maps = _prep_inputs(hidden, src, dst)
    key = (tuple(spec["caps"]),
           tuple(tuple(x) for x in spec["slot_lo"]),
           tuple(tuple(x) for x in spec["slot_w"]))
    if _cached.get("key") != key:
        _cached["nc"] = _build_program(spec)
        _cached["key"] = key
    nc = _cached["nc"]
    res = run_bass_kernel_spmd(nc, in_maps, core_ids=list(range(N_CORES)),
                               **run_kwargs)
    out = np.concatenate(
        [res.results[k]["y"][:DST_PER_CORE] for k in range(N_CORES)], axis=0)
    if run_kwargs:
        _cached["last_results"] = res
    return out
